# revision 16
# baseline (speedup 1.0000x reference)
"""BatchAllTripletLoss on 8 Trainium2 NeuronCores (sparsity version).

Contract: kernel(**inputs) takes the FULL inputs (embs [512,128] f32,
idtys [512] int64) and returns the FULL output (scalar f32 loss).

Math: d = pairwise euclidean distances [512,512];
  loss = sum_{a,p,n} relu(d[a,p]-d[a,n]+margin)*mask / (num_pos + eps)
The triplet mask factorizes as pos[a,p]*neg[a,n] (pos: same id, p!=a;
neg: different id). With 64 ids over 512 samples, each anchor has only
~8 valid positives, so instead of brute-forcing all 512 p columns we
enumerate, per anchor, the members of its id group (ranked by a
device-side counting argsort) and only process those columns:

 1. d rows for this core's 128 anchors via PE matmul (+sq rank-1 folds),
    dneg = d + BIG*same  (neg mask folded; pushes relu/count to 0).
 2. Group member table: rank R_i = #(j<i with id_j==id_i) via a fused
    is_lt*same row-reduce; scatter index i into a DRAM table at row
    id_i*32 + perm(R_i) (indirect DMA). perm rotates ranks so that THIS
    core's parity class (R%2 == core parity) lands in columns 0..15 --
    per-core variation rides in input data, the program stays SPMD.
 3. Gather each anchor's member row [128,32] (indirect DMA by id), then
    per k-column: gather member embeddings [128,128], rowdot -> d[a,p]
    via sqrt(sqA+sqP-2dot), x = (d+margin)*(valid & p!=a).
 4. Main loop over just 16 k-columns (vs 512 brute-force):
    ACT: t = relu(x - y) bf16; DVE: g = 1[y_bf16 < x] bf16; the PE
    reduces every tile with ones[128,1] matmuls accumulated into PSUM.
Per-core output [1,2] = (relu sum, count); host sums cores and divides.
"""

import numpy as np

B = 512
D = 128
NCORES = 8
AH = 128          # anchors per core
KMAX = 16         # member-table width (max group size supported)
KP = 8            # k-columns processed per core (rank-half split of KMAX)
MARGIN = 0.2
BIG = 1.0e6

_CACHE = {}


def _build_bass():
    import concourse.bass as bass
    import concourse.tile as tile
    from concourse import mybir

    f32 = mybir.dt.float32
    i32 = mybir.dt.int32
    bf16 = mybir.dt.bfloat16
    AF = mybir.ActivationFunctionType
    OP = mybir.AluOpType
    IOA = bass.IndirectOffsetOnAxis
    X = mybir.AxisListType.X

    nc = bass.Bass()

    emT = nc.dram_tensor("emT", [D, B], f32, kind="ExternalInput")     # embs.T
    emTA = nc.dram_tensor("emTA", [D, AH], f32, kind="ExternalInput")  # anchor cols
    rowb = nc.dram_tensor("rowb", [AH, 1], f32, kind="ExternalInput")  # 512*partition
    rows = nc.dram_tensor("rows", [1, 2 * B], f32, kind="ExternalInput")  # [ids|idx]
    colsA = nc.dram_tensor("colsA", [AH, 2], f32, kind="ExternalInput")  # idsA,idxA
    idsAll = nc.dram_tensor("idsAll", [AH, 4], f32, kind="ExternalInput")
    idxAll = nc.dram_tensor("idxAll", [AH, 4], f32, kind="ExternalInput")
    kidx = nc.dram_tensor("kidx", [AH, KP], f32, kind="ExternalInput")  # 8*par+j
    parc = nc.dram_tensor("parc", [AH, 1], f32, kind="ExternalInput")   # 8*par
    out = nc.dram_tensor("out", [1, 2], f32, kind="ExternalOutput")

    mtab = nc.dram_tensor("mtab", [64 * KMAX, 1], i32)  # member table scratch
    dchd = nc.dram_tensor("dchd", [AH * B, 1], f32)      # d rows staged for gather

    with tile.TileContext(nc) as tc:
        with (
            tc.tile_pool(name="sb", bufs=1) as sb,
            tc.tile_pool(name="psrow", bufs=1, space="PSUM") as psrow,
            tc.tile_pool(name="psbig", bufs=2, space="PSUM") as psbig,
            tc.tile_pool(name="psacc", bufs=1, space="PSUM") as psacc,
            tc.tile_pool(name="junka", bufs=4) as junka,
            tc.tile_pool(name="junkc", bufs=4) as junkc,
            tc.tile_pool(name="small", bufs=4) as small,
            tc.tile_pool(name="emb", bufs=4) as emb,
        ):
            # ---- load inputs
            emT_t = sb.tile([D, B], f32)
            emTA_t = sb.tile([D, AH], f32)
            rowb_t = sb.tile([AH, 1], f32)
            rows_t = sb.tile([1, 2 * B], f32)
            colsA_t = sb.tile([AH, 2], f32)
            idsAll_t = sb.tile([AH, 4], f32)
            idxAll_t = sb.tile([AH, 4], f32)
            kidx_t = sb.tile([AH, KP], f32)
            parc_t = sb.tile([AH, 1], f32)
            nc.sync.dma_start(out=emT_t[:], in_=emT[:])
            nc.sync.dma_start(out=emTA_t[:], in_=emTA[:])
            nc.sync.dma_start(out=rowb_t[:], in_=rowb[:])
            nc.sync.dma_start(out=rows_t[:], in_=rows[:])
            nc.sync.dma_start(out=colsA_t[:], in_=colsA[:])
            nc.sync.dma_start(out=idsAll_t[:], in_=idsAll[:])
            nc.sync.dma_start(out=idxAll_t[:], in_=idxAll[:])
            nc.sync.dma_start(out=kidx_t[:], in_=kidx[:])
            nc.sync.dma_start(out=parc_t[:], in_=parc[:])

            ones128 = sb.tile([D, 1], f32)
            nc.vector.memset(ones128[:], 1.0)
            ones128b = sb.tile([D, 1], bf16)
            nc.vector.memset(ones128b[:], 1.0)
            ones1 = sb.tile([1, D], f32)
            nc.vector.memset(ones1[:], 1.0)
            ones_row = sb.tile([1, B], f32)
            nc.vector.memset(ones_row[:], 1.0)

            # ---- squared norms
            sq_sb = sb.tile([1, B], f32)
            sqa_sb = sb.tile([1, AH], f32)
            e2 = sb.tile([D, B], f32)
            nc.vector.tensor_mul(e2[:], emT_t[:], emT_t[:])
            ps_sq = psrow.tile([1, B], f32, tag="row")
            nc.tensor.matmul(ps_sq[:], ones128[:], e2[:], start=True, stop=True)
            nc.scalar.copy(sq_sb[:], ps_sq[:])
            e2a = sb.tile([D, AH], f32)
            nc.vector.tensor_mul(e2a[:], emTA_t[:], emTA_t[:])
            ps_sqa = psrow.tile([1, AH], f32, tag="row")
            nc.tensor.matmul(ps_sqa[:], ones128[:], e2a[:], start=True, stop=True)
            nc.scalar.copy(sqa_sb[:], ps_sqa[:])

            emTAm2 = sb.tile([D, AH], f32)
            nc.vector.tensor_scalar_mul(emTAm2[:], emTA_t[:], -2.0)

            # d2 rows for this core's anchors, full n range
            ps_d2 = psbig.tile([AH, B], f32, tag="big")
            nc.tensor.matmul(ps_d2[:], emTAm2[:], emT_t[:], start=True, stop=False)
            nc.tensor.matmul(ps_d2[:], sqa_sb[:], ones_row[:], start=False, stop=False)
            nc.tensor.matmul(ps_d2[:], ones1[:, 0:AH], sq_sb[:], start=False, stop=True)
            d2r = sb.tile([AH, B], f32)
            nc.vector.tensor_scalar_max(d2r[:], ps_d2[:], 0.0)
            dch = sb.tile([AH, B], f32)
            nc.scalar.activation(dch[:], d2r[:], AF.Sqrt)
            dchd_v = dchd[:].rearrange("(a b) one -> a (b one)", a=AH)
            nc.sync.dma_start(out=dchd_v, in_=dch[:])

            # id/idx broadcast rows (persist through the scatter loop)
            ps_ids = psbig.tile([AH, B], f32, tag="big")
            nc.tensor.matmul(ps_ids[:], ones1[:], rows_t[0:1, 0:B], start=True, stop=True)
            ps_idx = psbig.tile([AH, B], f32, tag="big")
            nc.tensor.matmul(
                ps_idx[:], ones1[:], rows_t[0:1, B : 2 * B], start=True, stop=True
            )

            # dneg = d + BIG*same; group size cA per anchor
            s_full = sb.tile([AH, B], f32)
            nc.vector.tensor_scalar(
                out=s_full[:], in0=ps_ids[:], scalar1=colsA_t[:, 0:1], scalar2=None,
                op0=OP.is_equal,
            )
            dneg = sb.tile([AH, B], f32)
            nc.vector.scalar_tensor_tensor(
                out=dneg[:], in0=s_full[:], scalar=BIG, in1=dch[:],
                op0=OP.mult, op1=OP.add,
            )
            dneg_b = sb.tile([AH, B], bf16)
            nc.vector.tensor_copy(dneg_b[:], dneg[:])
            cA = sb.tile([AH, 1], f32)
            nc.vector.reduce_sum(cA[:], s_full[:], axis=X)

            # ---- build member table: scatter i -> mtab[id_i*32 + perm(R_i)]
            ztab = sb.tile([AH, (64 * KMAX) // AH], i32)  # [128,16]
            nc.vector.memset(ztab[:], 0)
            mtab_z = mtab[:].rearrange("(a b) one -> a (b one)", a=AH)
            nc.sync.dma_start(out=mtab_z, in_=ztab[:])

            for c4 in range(4):
                idc = idsAll_t[:, c4 : c4 + 1]
                ixc = idxAll_t[:, c4 : c4 + 1]
                s4 = small.tile([AH, B], f32, tag="s4")
                nc.vector.tensor_scalar(
                    out=s4[:], in0=ps_ids[:], scalar1=idc, scalar2=None,
                    op0=OP.is_equal,
                )
                jl = small.tile([AH, B], f32, tag="jl")
                nc.vector.tensor_scalar(
                    out=jl[:], in0=ps_idx[:], scalar1=ixc, scalar2=None,
                    op0=OP.is_lt,
                )
                jm = small.tile([AH, B], f32, tag="jm")
                nc.vector.tensor_mul(jm[:], jl[:], s4[:])
                r4 = small.tile([AH, 1], f32, tag="r4")
                jr = small.tile([AH, B], f32, tag="jr")
                nc.vector.tensor_scalar(
                    out=jr[:], in0=jm[:], scalar1=1.0, scalar2=None,
                    op0=OP.mult, op1=OP.add, accum_out=r4[:],
                )
                # perm: pos = R - 8*par + 16*[R < 8*par] -- rotates this
                # core's rank half to cols 0..7, parks the rest in 8..15
                w = small.tile([AH, 1], f32, tag="w")
                nc.vector.tensor_tensor(
                    out=w[:], in0=r4[:], in1=parc_t[:], op=OP.is_lt,
                )
                t16 = small.tile([AH, 1], f32, tag="t16")
                nc.vector.scalar_tensor_tensor(
                    out=t16[:], in0=w[:], scalar=16.0, in1=r4[:],
                    op0=OP.mult, op1=OP.add,
                )
                pos = small.tile([AH, 1], f32, tag="ps")
                nc.vector.tensor_sub(pos[:], t16[:], parc_t[:])
                o4 = small.tile([AH, 1], f32, tag="o4")
                nc.vector.scalar_tensor_tensor(
                    out=o4[:], in0=idc, scalar=float(KMAX), in1=pos[:],
                    op0=OP.mult, op1=OP.add,
                )
                o4i = small.tile([AH, 1], i32, tag="o4i")
                nc.vector.tensor_copy(o4i[:], o4[:])
                pay = small.tile([AH, 1], i32, tag="pay")
                nc.vector.tensor_copy(pay[:], ixc)
                nc.gpsimd.indirect_dma_start(
                    out=mtab[:], out_offset=IOA(ap=o4i[:, :1], axis=0),
                    in_=pay[:], in_offset=None,
                    bounds_check=64 * KMAX - 1, oob_is_err=False,
                )

            # gather each anchor's member row
            idsA_i = sb.tile([AH, 1], i32)
            nc.vector.tensor_copy(idsA_i[:], colsA_t[:, 0:1])
            ptab = sb.tile([AH, KMAX], i32)
            mtab_g = mtab[:].rearrange("(g k) one -> g (k one)", g=64)
            nc.gpsimd.indirect_dma_start(
                out=ptab[:], out_offset=None, in_=mtab_g,
                in_offset=IOA(ap=idsA_i[:, :1], axis=0),
            )
            pf = sb.tile([AH, KMAX], f32)
            nc.vector.tensor_copy(pf[:], ptab[:])
            selfm = sb.tile([AH, KP], f32)
            nc.vector.tensor_scalar(
                out=selfm[:], in0=pf[:, 0:KP], scalar1=colsA_t[:, 1:2], scalar2=None,
                op0=OP.is_equal,
            )
            kv = sb.tile([AH, KP], f32)
            nc.vector.tensor_scalar(
                out=kv[:], in0=kidx_t[:], scalar1=cA[:], scalar2=None, op0=OP.is_lt,
            )
            vm = sb.tile([AH, KP], f32)
            nc.vector.tensor_sub(vm[:], kv[:], selfm[:])

            # ---- fetch d[a, p] for every member column in one gather
            pfs = sb.tile([AH, KP], f32)
            nc.vector.tensor_scalar(
                out=pfs[:], in0=pf[:, 0:KP], scalar1=rowb_t[:, 0:1], scalar2=None,
                op0=OP.add,
            )
            offi = sb.tile([AH, KP], i32)
            nc.vector.tensor_copy(offi[:], pfs[:])
            xg = sb.tile([AH, KP], f32)
            for j in range(KP):
                nc.gpsimd.indirect_dma_start(
                    out=xg[:, j : j + 1], out_offset=None, in_=dchd[:],
                    in_offset=IOA(ap=offi[:, j : j + 1], axis=0),
                )
            djm = sb.tile([AH, KP], f32)
            nc.vector.tensor_scalar_add(djm[:], xg[:], MARGIN)
            xall = sb.tile([AH, KP], f32)
            nc.vector.tensor_mul(xall[:], djm[:], vm[:])

            # ---- main loop over KP member columns
            ps_relu = psacc.tile([1, B], f32)
            ps_cnt = psacc.tile([1, B], f32)

            for j in range(KP):
                xj = xall[:, j : j + 1]
                t = junka.tile([AH, B], bf16)
                nc.scalar.activation(t[:], dneg[:], AF.Relu, bias=xj[:], scale=-1.0)
                nc.tensor.matmul(
                    ps_relu[:], ones128b[:], t[:],
                    start=(j == 0), stop=(j == KP - 1),
                )
                g = junkc.tile([AH, B], bf16)
                nc.vector.tensor_scalar(
                    out=g[:], in0=dneg_b[:], scalar1=xj[:], scalar2=None, op0=OP.is_lt,
                )
                nc.tensor.matmul(
                    ps_cnt[:], ones128b[:], g[:],
                    start=(j == 0), stop=(j == KP - 1),
                )

            # ---- final
            res = sb.tile([1, 2], f32)
            nc.vector.reduce_sum(res[:, 0:1], ps_relu[:], axis=X)
            nc.vector.reduce_sum(res[:, 1:2], ps_cnt[:], axis=X)
            nc.sync.dma_start(out=out[:], in_=res[:])

    return nc


def _legalize_waits(bir: bytes) -> bytes:
    """walrus codegen in this toolchain allows only one sync-wait per
    instruction; split extra waits into standalone EventSemaphore insts."""
    import json

    m = json.loads(bir)
    for fn in m["functions"]:
        for bb in fn["blocks"]:
            new = []
            for inst in bb["instructions"]:
                si = inst.get("sync_info")
                if si and si.get("on_wait") and len(si["on_wait"]) > 1:
                    waits = si["on_wait"]
                    for j, w in enumerate(waits[:-1]):
                        new.append(
                            {
                                "engine": inst["engine"],
                                "ins": [],
                                "outs": [],
                                "name": f"{inst['name']}-w{j}",
                                "opcode": "EventSemaphore",
                                "sync_info": {"on_update": [], "on_wait": [w]},
                            }
                        )
                    si["on_wait"] = [waits[-1]]
                new.append(inst)
            bb["instructions"] = new
    return json.dumps(m).encode()


def _get_nc():
    if "nc" not in _CACHE:
        nc = _build_bass()
        orig = nc.to_json_bytes
        nc.to_json_bytes = lambda: _legalize_waits(orig())
        _CACHE["nc"] = nc
    return _CACHE["nc"]


def make_in_maps(embs: np.ndarray, idtys: np.ndarray):
    embs = np.ascontiguousarray(np.asarray(embs, dtype=np.float32))
    emT = np.ascontiguousarray(embs.T)  # [D, B]
    ids = np.asarray(idtys).astype(np.float32)
    idx = np.arange(B, dtype=np.float32)
    in_maps = []
    for c in range(NCORES):
        a0 = (c // 2) * AH
        par = c % 2
        rows = np.concatenate([ids, idx])[None, :]
        colsA = np.stack([ids[a0 : a0 + AH], idx[a0 : a0 + AH]], axis=1)
        kcol = (np.arange(KP, dtype=np.float32) + 8.0 * par)[None, :]
        in_maps.append(
            {
                "emT": emT,
                "emTA": np.ascontiguousarray(emT[:, a0 : a0 + AH]),
                "rowb": (np.arange(AH, dtype=np.float32) * B).reshape(AH, 1),
                "rows": np.ascontiguousarray(rows.astype(np.float32)),
                "colsA": np.ascontiguousarray(colsA.astype(np.float32)),
                "idsAll": np.ascontiguousarray(ids.reshape(4, AH).T),
                "idxAll": np.ascontiguousarray(idx.reshape(4, AH).T),
                "kidx": np.ascontiguousarray(np.repeat(kcol, AH, axis=0)),
                "parc": np.full((AH, 1), 8.0 * par, dtype=np.float32),
            }
        )
    return in_maps


def combine(results):
    total = 0.0
    count = 0.0
    for r in results:
        o = np.asarray(r["out"], dtype=np.float64)
        total += o[0, 0]
        count += o[0, 1]
    loss = np.float32(total / (count + 1e-16))
    return np.array(loss, dtype=np.float32)


def kernel(embs: np.ndarray, idtys: np.ndarray) -> np.ndarray:
    from concourse import bass_utils

    nc = _get_nc()
    in_maps = make_in_maps(np.asarray(embs), np.asarray(idtys))
    res = bass_utils.run_bass_kernel_spmd(nc, in_maps, list(range(NCORES)))
    return combine(res.results)


# revision 17
# speedup vs baseline: 1.0539x; 1.0539x over previous
"""BatchAllTripletLoss on 8 Trainium2 NeuronCores (sparsity version).

Contract: kernel(**inputs) takes the FULL inputs (embs [512,128] f32,
idtys [512] int64) and returns the FULL output (scalar f32 loss).

Math: d = pairwise euclidean distances [512,512];
  loss = sum_{a,p,n} relu(d[a,p]-d[a,n]+margin)*mask / (num_pos + eps)
The triplet mask factorizes as pos[a,p]*neg[a,n] (pos: same id, p!=a;
neg: different id). With 64 ids over 512 samples, each anchor has only
~8 valid positives, so instead of brute-forcing all 512 p columns we
enumerate, per anchor, the members of its id group (ranked by a
device-side counting argsort) and only process those columns:

 1. d rows for this core's 128 anchors via PE matmul (+sq rank-1 folds),
    dneg = d + BIG*same  (neg mask folded; pushes relu/count to 0).
 2. Group member table: rank R_i = #(j<i with id_j==id_i) via a fused
    is_lt*same row-reduce; scatter index i into a DRAM table at row
    id_i*32 + perm(R_i) (indirect DMA). perm rotates ranks so that THIS
    core's parity class (R%2 == core parity) lands in columns 0..15 --
    per-core variation rides in input data, the program stays SPMD.
 3. Gather each anchor's member row [128,32] (indirect DMA by id), then
    per k-column: gather member embeddings [128,128], rowdot -> d[a,p]
    via sqrt(sqA+sqP-2dot), x = (d+margin)*(valid & p!=a).
 4. Main loop over just 16 k-columns (vs 512 brute-force):
    ACT: t = relu(x - y) bf16; DVE: g = 1[y_bf16 < x] bf16; the PE
    reduces every tile with ones[128,1] matmuls accumulated into PSUM.
Per-core output [1,2] = (relu sum, count); host sums cores and divides.
"""

import numpy as np

B = 512
D = 128
NCORES = 8
AH = 128          # anchors per core
KMAX = 16         # member-table width (max group size supported)
KP = 8            # k-columns processed per core (rank-half split of KMAX)
MARGIN = 0.2
BIG = 1.0e6

_CACHE = {}


def _build_bass():
    import concourse.bass as bass
    import concourse.tile as tile
    from concourse import mybir

    f32 = mybir.dt.float32
    i32 = mybir.dt.int32
    bf16 = mybir.dt.bfloat16
    AF = mybir.ActivationFunctionType
    OP = mybir.AluOpType
    IOA = bass.IndirectOffsetOnAxis
    X = mybir.AxisListType.X

    nc = bass.Bass()

    emT = nc.dram_tensor("emT", [D, B], f32, kind="ExternalInput")     # embs.T
    emTA = nc.dram_tensor("emTA", [D, AH], f32, kind="ExternalInput")  # anchor cols
    rowb = nc.dram_tensor("rowb", [AH, 1], f32, kind="ExternalInput")  # 512*partition
    rows = nc.dram_tensor("rows", [1, 2 * B], f32, kind="ExternalInput")  # [ids|idx]
    colsA = nc.dram_tensor("colsA", [AH, 2], f32, kind="ExternalInput")  # idsA,idxA
    idsAll = nc.dram_tensor("idsAll", [AH, 4], f32, kind="ExternalInput")
    idxAll = nc.dram_tensor("idxAll", [AH, 4], f32, kind="ExternalInput")
    kidx = nc.dram_tensor("kidx", [AH, KP], f32, kind="ExternalInput")  # 8*par+j
    parc = nc.dram_tensor("parc", [AH, 1], f32, kind="ExternalInput")   # 8*par
    out = nc.dram_tensor("out", [1, 2], f32, kind="ExternalOutput")

    mtab = nc.dram_tensor("mtab", [64 * KMAX, 1], i32)  # member table scratch
    dchd = nc.dram_tensor("dchd", [AH * B, 1], f32)      # d rows staged for gather

    with tile.TileContext(nc) as tc:
        with (
            tc.tile_pool(name="sb", bufs=1) as sb,
            tc.tile_pool(name="psrow", bufs=1, space="PSUM") as psrow,
            tc.tile_pool(name="psbig", bufs=2, space="PSUM") as psbig,
            tc.tile_pool(name="psacc", bufs=1, space="PSUM") as psacc,
            tc.tile_pool(name="junka", bufs=4) as junka,
            tc.tile_pool(name="junkc", bufs=4) as junkc,
            tc.tile_pool(name="small", bufs=4) as small,
            tc.tile_pool(name="emb", bufs=4) as emb,
        ):
            # ---- load inputs
            emT_t = sb.tile([D, B], f32)
            emTA_t = sb.tile([D, AH], f32)
            rowb_t = sb.tile([AH, 1], f32)
            rows_t = sb.tile([1, 2 * B], f32)
            colsA_t = sb.tile([AH, 2], f32)
            idsAll_t = sb.tile([AH, 4], f32)
            idxAll_t = sb.tile([AH, 4], f32)
            kidx_t = sb.tile([AH, KP], f32)
            parc_t = sb.tile([AH, 1], f32)
            nc.sync.dma_start(out=emT_t[:], in_=emT[:])
            nc.sync.dma_start(out=emTA_t[:], in_=emTA[:])
            nc.sync.dma_start(out=rowb_t[:], in_=rowb[:])
            nc.sync.dma_start(out=rows_t[:], in_=rows[:])
            nc.sync.dma_start(out=colsA_t[:], in_=colsA[:])
            nc.sync.dma_start(out=idsAll_t[:], in_=idsAll[:])
            nc.sync.dma_start(out=idxAll_t[:], in_=idxAll[:])
            nc.sync.dma_start(out=kidx_t[:], in_=kidx[:])
            nc.sync.dma_start(out=parc_t[:], in_=parc[:])

            ones128 = sb.tile([D, 1], f32)
            nc.vector.memset(ones128[:], 1.0)
            ones128b = sb.tile([D, 1], bf16)
            nc.vector.memset(ones128b[:], 1.0)
            ones1 = sb.tile([1, D], f32)
            nc.vector.memset(ones1[:], 1.0)
            ones_row = sb.tile([1, B], f32)
            nc.vector.memset(ones_row[:], 1.0)

            # ---- squared norms
            sq_sb = sb.tile([1, B], f32)
            sqa_sb = sb.tile([1, AH], f32)
            e2 = sb.tile([D, B], f32)
            nc.vector.tensor_mul(e2[:], emT_t[:], emT_t[:])
            ps_sq = psrow.tile([1, B], f32, tag="row")
            nc.tensor.matmul(ps_sq[:], ones128[:], e2[:], start=True, stop=True)
            nc.scalar.copy(sq_sb[:], ps_sq[:])
            e2a = sb.tile([D, AH], f32)
            nc.vector.tensor_mul(e2a[:], emTA_t[:], emTA_t[:])
            ps_sqa = psrow.tile([1, AH], f32, tag="row")
            nc.tensor.matmul(ps_sqa[:], ones128[:], e2a[:], start=True, stop=True)
            nc.scalar.copy(sqa_sb[:], ps_sqa[:])

            emTAm2 = sb.tile([D, AH], f32)
            nc.vector.tensor_scalar_mul(emTAm2[:], emTA_t[:], -2.0)

            # d2 rows for this core's anchors, full n range
            ps_d2 = psbig.tile([AH, B], f32, tag="big")
            nc.tensor.matmul(ps_d2[:], emTAm2[:], emT_t[:], start=True, stop=False)
            nc.tensor.matmul(ps_d2[:], sqa_sb[:], ones_row[:], start=False, stop=False)
            nc.tensor.matmul(ps_d2[:], ones1[:, 0:AH], sq_sb[:], start=False, stop=True)
            d2r = sb.tile([AH, B], f32)
            nc.vector.tensor_scalar_max(d2r[:], ps_d2[:], 0.0)
            dch = sb.tile([AH, B], f32)
            nc.scalar.activation(dch[:], d2r[:], AF.Sqrt)
            dchd_v = dchd[:].rearrange("(a b) one -> a (b one)", a=AH)
            nc.sync.dma_start(out=dchd_v, in_=dch[:])

            # id/idx broadcast rows (persist through the scatter loop)
            ps_ids = psbig.tile([AH, B], f32, tag="big")
            nc.tensor.matmul(ps_ids[:], ones1[:], rows_t[0:1, 0:B], start=True, stop=True)
            ps_idx = psbig.tile([AH, B], f32, tag="big")
            nc.tensor.matmul(
                ps_idx[:], ones1[:], rows_t[0:1, B : 2 * B], start=True, stop=True
            )

            # dneg = d + BIG*same; group size cA per anchor
            s_full = sb.tile([AH, B], f32)
            nc.vector.tensor_scalar(
                out=s_full[:], in0=ps_ids[:], scalar1=colsA_t[:, 0:1], scalar2=None,
                op0=OP.is_equal,
            )
            dneg = sb.tile([AH, B], f32)
            nc.vector.scalar_tensor_tensor(
                out=dneg[:], in0=s_full[:], scalar=BIG, in1=dch[:],
                op0=OP.mult, op1=OP.add,
            )
            dneg_b = sb.tile([AH, B], bf16)
            nc.vector.tensor_copy(dneg_b[:], dneg[:])
            cA = sb.tile([AH, 1], f32)
            nc.vector.reduce_sum(cA[:], s_full[:], axis=X)

            # ---- build member table: scatter i -> mtab[id_i*32 + perm(R_i)]
            ztab = sb.tile([AH, (64 * KMAX) // AH], i32)  # [128,16]
            nc.vector.memset(ztab[:], 0)
            mtab_z = mtab[:].rearrange("(a b) one -> a (b one)", a=AH)
            nc.sync.dma_start(out=mtab_z, in_=ztab[:])

            for c4 in range(4):
                idc = idsAll_t[:, c4 : c4 + 1]
                ixc = idxAll_t[:, c4 : c4 + 1]
                s4 = small.tile([AH, B], f32, tag="s4")
                nc.vector.tensor_scalar(
                    out=s4[:], in0=ps_ids[:], scalar1=idc, scalar2=None,
                    op0=OP.is_equal,
                )
                jl = small.tile([AH, B], f32, tag="jl")
                nc.vector.tensor_scalar(
                    out=jl[:], in0=ps_idx[:], scalar1=ixc, scalar2=None,
                    op0=OP.is_lt,
                )
                jm = small.tile([AH, B], f32, tag="jm")
                nc.vector.tensor_mul(jm[:], jl[:], s4[:])
                r4 = small.tile([AH, 1], f32, tag="r4")
                jr = small.tile([AH, B], f32, tag="jr")
                nc.vector.tensor_scalar(
                    out=jr[:], in0=jm[:], scalar1=1.0, scalar2=None,
                    op0=OP.mult, op1=OP.add, accum_out=r4[:],
                )
                # perm: pos = R - 8*par + 16*[R < 8*par] -- rotates this
                # core's rank half to cols 0..7, parks the rest in 8..15
                w = small.tile([AH, 1], f32, tag="w")
                nc.vector.tensor_tensor(
                    out=w[:], in0=r4[:], in1=parc_t[:], op=OP.is_lt,
                )
                t16 = small.tile([AH, 1], f32, tag="t16")
                nc.vector.scalar_tensor_tensor(
                    out=t16[:], in0=w[:], scalar=16.0, in1=r4[:],
                    op0=OP.mult, op1=OP.add,
                )
                pos = small.tile([AH, 1], f32, tag="ps")
                nc.vector.tensor_sub(pos[:], t16[:], parc_t[:])
                o4 = small.tile([AH, 1], f32, tag="o4")
                nc.vector.scalar_tensor_tensor(
                    out=o4[:], in0=idc, scalar=float(KMAX), in1=pos[:],
                    op0=OP.mult, op1=OP.add,
                )
                o4i = small.tile([AH, 1], i32, tag="o4i")
                nc.vector.tensor_copy(o4i[:], o4[:])
                pay = small.tile([AH, 1], i32, tag="pay")
                nc.vector.tensor_copy(pay[:], ixc)
                nc.gpsimd.indirect_dma_start(
                    out=mtab[:], out_offset=IOA(ap=o4i[:, :1], axis=0),
                    in_=pay[:], in_offset=None,
                    bounds_check=64 * KMAX - 1, oob_is_err=False,
                )

            # gather each anchor's member row
            idsA_i = sb.tile([AH, 1], i32)
            nc.vector.tensor_copy(idsA_i[:], colsA_t[:, 0:1])
            ptab = sb.tile([AH, KMAX], i32)
            mtab_g = mtab[:].rearrange("(g k) one -> g (k one)", g=64)
            nc.gpsimd.indirect_dma_start(
                out=ptab[:], out_offset=None, in_=mtab_g,
                in_offset=IOA(ap=idsA_i[:, :1], axis=0),
            )
            pf = sb.tile([AH, KMAX], f32)
            nc.vector.tensor_copy(pf[:], ptab[:])
            selfm = sb.tile([AH, KP], f32)
            nc.vector.tensor_scalar(
                out=selfm[:], in0=pf[:, 0:KP], scalar1=colsA_t[:, 1:2], scalar2=None,
                op0=OP.is_equal,
            )
            kv = sb.tile([AH, KP], f32)
            nc.vector.tensor_scalar(
                out=kv[:], in0=kidx_t[:], scalar1=cA[:], scalar2=None, op0=OP.is_lt,
            )
            vm = sb.tile([AH, KP], f32)
            nc.vector.tensor_sub(vm[:], kv[:], selfm[:])

            # ---- fetch d[a, p] for every member column in one gather
            pfs = sb.tile([AH, KP], f32)
            nc.vector.tensor_scalar(
                out=pfs[:], in0=pf[:, 0:KP], scalar1=rowb_t[:, 0:1], scalar2=None,
                op0=OP.add,
            )
            offi = sb.tile([AH, KP], i32)
            nc.vector.tensor_copy(offi[:], pfs[:])
            xg = sb.tile([AH, KP], f32)
            xall = sb.tile([AH, KP], f32)

            # ---- main loop over KP member columns (gather -> mask -> ops
            # per column so the pipeline fills column by column)
            ps_relu = psacc.tile([1, B], f32)
            ps_cnt = psacc.tile([1, B], f32)

            for j in range(KP):
                nc.gpsimd.indirect_dma_start(
                    out=xg[:, j : j + 1], out_offset=None, in_=dchd[:],
                    in_offset=IOA(ap=offi[:, j : j + 1], axis=0),
                )
                djm = small.tile([AH, 1], f32, tag="djm")
                nc.vector.tensor_scalar_add(djm[:], xg[:, j : j + 1], MARGIN)
                nc.vector.tensor_mul(xall[:, j : j + 1], djm[:], vm[:, j : j + 1])
                xj = xall[:, j : j + 1]
                t = junka.tile([AH, B], bf16)
                nc.scalar.activation(t[:], dneg[:], AF.Relu, bias=xj[:], scale=-1.0)
                nc.tensor.matmul(
                    ps_relu[:], ones128b[:], t[:],
                    start=(j == 0), stop=(j == KP - 1),
                )
                g = junkc.tile([AH, B], bf16)
                nc.vector.tensor_scalar(
                    out=g[:], in0=dneg_b[:], scalar1=xj[:], scalar2=None, op0=OP.is_lt,
                )
                nc.tensor.matmul(
                    ps_cnt[:], ones128b[:], g[:],
                    start=(j == 0), stop=(j == KP - 1),
                )

            # ---- final
            res = sb.tile([1, 2], f32)
            nc.vector.reduce_sum(res[:, 0:1], ps_relu[:], axis=X)
            nc.vector.reduce_sum(res[:, 1:2], ps_cnt[:], axis=X)
            nc.sync.dma_start(out=out[:], in_=res[:])

    return nc


def _legalize_waits(bir: bytes) -> bytes:
    """walrus codegen in this toolchain allows only one sync-wait per
    instruction; split extra waits into standalone EventSemaphore insts."""
    import json

    m = json.loads(bir)
    for fn in m["functions"]:
        for bb in fn["blocks"]:
            new = []
            for inst in bb["instructions"]:
                si = inst.get("sync_info")
                if si and si.get("on_wait") and len(si["on_wait"]) > 1:
                    waits = si["on_wait"]
                    for j, w in enumerate(waits[:-1]):
                        new.append(
                            {
                                "engine": inst["engine"],
                                "ins": [],
                                "outs": [],
                                "name": f"{inst['name']}-w{j}",
                                "opcode": "EventSemaphore",
                                "sync_info": {"on_update": [], "on_wait": [w]},
                            }
                        )
                    si["on_wait"] = [waits[-1]]
                new.append(inst)
            bb["instructions"] = new
    return json.dumps(m).encode()


def _get_nc():
    if "nc" not in _CACHE:
        nc = _build_bass()
        orig = nc.to_json_bytes
        nc.to_json_bytes = lambda: _legalize_waits(orig())
        _CACHE["nc"] = nc
    return _CACHE["nc"]


def make_in_maps(embs: np.ndarray, idtys: np.ndarray):
    embs = np.ascontiguousarray(np.asarray(embs, dtype=np.float32))
    emT = np.ascontiguousarray(embs.T)  # [D, B]
    ids = np.asarray(idtys).astype(np.float32)
    idx = np.arange(B, dtype=np.float32)
    in_maps = []
    for c in range(NCORES):
        a0 = (c // 2) * AH
        par = c % 2
        rows = np.concatenate([ids, idx])[None, :]
        colsA = np.stack([ids[a0 : a0 + AH], idx[a0 : a0 + AH]], axis=1)
        kcol = (np.arange(KP, dtype=np.float32) + 8.0 * par)[None, :]
        in_maps.append(
            {
                "emT": emT,
                "emTA": np.ascontiguousarray(emT[:, a0 : a0 + AH]),
                "rowb": (np.arange(AH, dtype=np.float32) * B).reshape(AH, 1),
                "rows": np.ascontiguousarray(rows.astype(np.float32)),
                "colsA": np.ascontiguousarray(colsA.astype(np.float32)),
                "idsAll": np.ascontiguousarray(ids.reshape(4, AH).T),
                "idxAll": np.ascontiguousarray(idx.reshape(4, AH).T),
                "kidx": np.ascontiguousarray(np.repeat(kcol, AH, axis=0)),
                "parc": np.full((AH, 1), 8.0 * par, dtype=np.float32),
            }
        )
    return in_maps


def combine(results):
    total = 0.0
    count = 0.0
    for r in results:
        o = np.asarray(r["out"], dtype=np.float64)
        total += o[0, 0]
        count += o[0, 1]
    loss = np.float32(total / (count + 1e-16))
    return np.array(loss, dtype=np.float32)


def kernel(embs: np.ndarray, idtys: np.ndarray) -> np.ndarray:
    from concourse import bass_utils

    nc = _get_nc()
    in_maps = make_in_maps(np.asarray(embs), np.asarray(idtys))
    res = bass_utils.run_bass_kernel_spmd(nc, in_maps, list(range(NCORES)))
    return combine(res.results)


# revision 18
# speedup vs baseline: 1.1647x; 1.1052x over previous
"""BatchAllTripletLoss on 8 Trainium2 NeuronCores (sparsity version).

Contract: kernel(**inputs) takes the FULL inputs (embs [512,128] f32,
idtys [512] int64) and returns the FULL output (scalar f32 loss).

Math: d = pairwise euclidean distances [512,512];
  loss = sum_{a,p,n} relu(d[a,p]-d[a,n]+margin)*mask / (num_pos + eps)
The triplet mask factorizes as pos[a,p]*neg[a,n] (pos: same id, p!=a;
neg: different id). With 64 ids over 512 samples, each anchor has only
~8 valid positives, so instead of brute-forcing all 512 p columns we
enumerate, per anchor, the members of its id group (ranked by a
device-side counting argsort) and only process those columns:

 1. d rows for this core's 128 anchors via PE matmul (+sq rank-1 folds),
    dneg = d + BIG*same  (neg mask folded; pushes relu/count to 0).
 2. Group member table: rank R_i = #(j<i with id_j==id_i) via a fused
    is_lt*same row-reduce; scatter index i into a DRAM table at row
    id_i*32 + perm(R_i) (indirect DMA). perm rotates ranks so that THIS
    core's parity class (R%2 == core parity) lands in columns 0..15 --
    per-core variation rides in input data, the program stays SPMD.
 3. Gather each anchor's member row [128,32] (indirect DMA by id), then
    per k-column: gather member embeddings [128,128], rowdot -> d[a,p]
    via sqrt(sqA+sqP-2dot), x = (d+margin)*(valid & p!=a).
 4. Main loop over just 16 k-columns (vs 512 brute-force):
    ACT: t = relu(x - y) bf16; DVE: g = 1[y_bf16 < x] bf16; the PE
    reduces every tile with ones[128,1] matmuls accumulated into PSUM.
Per-core output [1,2] = (relu sum, count); host sums cores and divides.
"""

import numpy as np

B = 512
D = 128
NCORES = 8
AH = 128          # anchors per core
KMAX = 16         # member-table width (max group size supported)
KP = 8            # k-columns processed per core (rank-half split of KMAX)
MARGIN = 0.2
BIG = 1.0e6

_CACHE = {}


def _build_bass():
    import concourse.bass as bass
    import concourse.tile as tile
    from concourse import mybir

    f32 = mybir.dt.float32
    i32 = mybir.dt.int32
    bf16 = mybir.dt.bfloat16
    AF = mybir.ActivationFunctionType
    OP = mybir.AluOpType
    IOA = bass.IndirectOffsetOnAxis
    X = mybir.AxisListType.X

    nc = bass.Bass()

    emT = nc.dram_tensor("emT", [D, B], f32, kind="ExternalInput")     # embs.T
    emTA = nc.dram_tensor("emTA", [D, AH], f32, kind="ExternalInput")  # anchor cols
    rowb = nc.dram_tensor("rowb", [AH, 1], f32, kind="ExternalInput")  # 512*partition
    rows = nc.dram_tensor("rows", [1, 2 * B], f32, kind="ExternalInput")  # [ids|idx]
    colsA = nc.dram_tensor("colsA", [AH, 2], f32, kind="ExternalInput")  # idsA,idxA
    idsAll = nc.dram_tensor("idsAll", [AH, 4], f32, kind="ExternalInput")
    idxAll = nc.dram_tensor("idxAll", [AH, 4], f32, kind="ExternalInput")
    kidx = nc.dram_tensor("kidx", [AH, KP], f32, kind="ExternalInput")  # 8*par+j
    parc = nc.dram_tensor("parc", [AH, 1], f32, kind="ExternalInput")   # 8*par
    idsAr = nc.dram_tensor("idsAr", [1, AH], f32, kind="ExternalInput")  # ids of anchors, row
    gcol = nc.dram_tensor("gcol", [64, 1], f32, kind="ExternalInput")    # 0..63
    out = nc.dram_tensor("out", [1, 2], f32, kind="ExternalOutput")

    dchd = nc.dram_tensor("dchd", [AH * B, 1], f32)      # d rows staged for gather

    with tile.TileContext(nc) as tc:
        with (
            tc.tile_pool(name="sb", bufs=1) as sb,
            tc.tile_pool(name="psrow", bufs=1, space="PSUM") as psrow,
            tc.tile_pool(name="psbig", bufs=2, space="PSUM") as psbig,
            tc.tile_pool(name="psacc", bufs=1, space="PSUM") as psacc,
            tc.tile_pool(name="junka", bufs=4) as junka,
            tc.tile_pool(name="junkc", bufs=4) as junkc,
            tc.tile_pool(name="small", bufs=4) as small,
            tc.tile_pool(name="emb", bufs=4) as emb,
        ):
            # ---- load inputs
            emT_t = sb.tile([D, B], f32)
            emTA_t = sb.tile([D, AH], f32)
            rowb_t = sb.tile([AH, 1], f32)
            rows_t = sb.tile([1, 2 * B], f32)
            colsA_t = sb.tile([AH, 2], f32)
            idsAll_t = sb.tile([AH, 4], f32)
            idxAll_t = sb.tile([AH, 4], f32)
            kidx_t = sb.tile([AH, KP], f32)
            parc_t = sb.tile([AH, 1], f32)
            idsAr_t = sb.tile([1, AH], f32)
            gcol_t = sb.tile([64, 1], f32)
            nc.sync.dma_start(out=emT_t[:], in_=emT[:])
            nc.sync.dma_start(out=emTA_t[:], in_=emTA[:])
            nc.sync.dma_start(out=rowb_t[:], in_=rowb[:])
            nc.sync.dma_start(out=rows_t[:], in_=rows[:])
            nc.sync.dma_start(out=colsA_t[:], in_=colsA[:])
            nc.sync.dma_start(out=idsAll_t[:], in_=idsAll[:])
            nc.sync.dma_start(out=idxAll_t[:], in_=idxAll[:])
            nc.sync.dma_start(out=kidx_t[:], in_=kidx[:])
            nc.sync.dma_start(out=parc_t[:], in_=parc[:])
            nc.sync.dma_start(out=idsAr_t[:], in_=idsAr[:])
            nc.sync.dma_start(out=gcol_t[:], in_=gcol[:])

            ones128 = sb.tile([D, 1], f32)
            nc.vector.memset(ones128[:], 1.0)
            ones128b = sb.tile([D, 1], bf16)
            nc.vector.memset(ones128b[:], 1.0)
            ones1 = sb.tile([1, D], f32)
            nc.vector.memset(ones1[:], 1.0)
            ones_row = sb.tile([1, B], f32)
            nc.vector.memset(ones_row[:], 1.0)

            # ---- squared norms
            sq_sb = sb.tile([1, B], f32)
            sqa_sb = sb.tile([1, AH], f32)
            e2 = sb.tile([D, B], f32)
            nc.vector.tensor_mul(e2[:], emT_t[:], emT_t[:])
            ps_sq = psrow.tile([1, B], f32, tag="row")
            nc.tensor.matmul(ps_sq[:], ones128[:], e2[:], start=True, stop=True)
            nc.scalar.copy(sq_sb[:], ps_sq[:])
            e2a = sb.tile([D, AH], f32)
            nc.vector.tensor_mul(e2a[:], emTA_t[:], emTA_t[:])
            ps_sqa = psrow.tile([1, AH], f32, tag="row")
            nc.tensor.matmul(ps_sqa[:], ones128[:], e2a[:], start=True, stop=True)
            nc.scalar.copy(sqa_sb[:], ps_sqa[:])

            emTAm2 = sb.tile([D, AH], f32)
            nc.vector.tensor_scalar_mul(emTAm2[:], emTA_t[:], -2.0)

            # d2 rows for this core's anchors, full n range
            ps_d2 = psbig.tile([AH, B], f32, tag="big")
            nc.tensor.matmul(ps_d2[:], emTAm2[:], emT_t[:], start=True, stop=False)
            nc.tensor.matmul(ps_d2[:], sqa_sb[:], ones_row[:], start=False, stop=False)
            nc.tensor.matmul(ps_d2[:], ones1[:, 0:AH], sq_sb[:], start=False, stop=True)
            d2r = sb.tile([AH, B], f32)
            nc.vector.tensor_scalar_max(d2r[:], ps_d2[:], 0.0)
            dch = sb.tile([AH, B], f32)
            nc.scalar.activation(dch[:], d2r[:], AF.Sqrt)
            dchd_v = dchd[:].rearrange("(a b) one -> a (b one)", a=AH)
            nc.sync.dma_start(out=dchd_v, in_=dch[:])

            # id/idx broadcast rows (persist through the scatter loop)
            ps_ids = psbig.tile([AH, B], f32, tag="big")
            nc.tensor.matmul(ps_ids[:], ones1[:], rows_t[0:1, 0:B], start=True, stop=True)
            ps_idx = psbig.tile([AH, B], f32, tag="big")
            nc.tensor.matmul(
                ps_idx[:], ones1[:], rows_t[0:1, B : 2 * B], start=True, stop=True
            )

            # dneg = d + BIG*same; group size cA per anchor
            s_full = sb.tile([AH, B], f32)
            nc.vector.tensor_scalar(
                out=s_full[:], in0=ps_ids[:], scalar1=colsA_t[:, 0:1], scalar2=None,
                op0=OP.is_equal,
            )
            dneg = sb.tile([AH, B], f32)
            nc.vector.scalar_tensor_tensor(
                out=dneg[:], in0=s_full[:], scalar=BIG, in1=dch[:],
                op0=OP.mult, op1=OP.add,
            )
            dneg_b = sb.tile([AH, B], bf16)
            nc.vector.tensor_copy(dneg_b[:], dneg[:])
            cA = sb.tile([AH, 1], f32)
            nc.vector.reduce_sum(cA[:], s_full[:], axis=X)

            # ---- build member table M[g,k] = sum_i id-onehot * pos-onehot * i
            # via PE matmuls over 4 chunks of i, then ptab[a,:] = M[id_a,:]
            # via a second one-hot matmul. No indirect DMA needed.
            ps_mt = psrow.tile([64, KMAX], f32, tag="mt")
            for c4 in range(4):
                idc = idsAll_t[:, c4 : c4 + 1]
                ixc = idxAll_t[:, c4 : c4 + 1]
                s4 = small.tile([AH, B], f32, tag="s4")
                nc.vector.tensor_scalar(
                    out=s4[:], in0=ps_ids[:], scalar1=idc, scalar2=None,
                    op0=OP.is_equal,
                )
                jl = small.tile([AH, B], f32, tag="jl")
                nc.vector.tensor_scalar(
                    out=jl[:], in0=ps_idx[:], scalar1=ixc, scalar2=None,
                    op0=OP.is_lt,
                )
                jm = small.tile([AH, B], f32, tag="jm")
                nc.vector.tensor_mul(jm[:], jl[:], s4[:])
                r4 = small.tile([AH, 1], f32, tag="r4")
                jr = small.tile([AH, B], f32, tag="jr")
                nc.vector.tensor_scalar(
                    out=jr[:], in0=jm[:], scalar1=1.0, scalar2=None,
                    op0=OP.mult, op1=OP.add, accum_out=r4[:],
                )
                # perm: pos = R - 8*par + 16*[R < 8*par] -- rotates this
                # core's rank half to cols 0..7, parks the rest in 8..15
                w = small.tile([AH, 1], f32, tag="w")
                nc.vector.tensor_tensor(
                    out=w[:], in0=r4[:], in1=parc_t[:], op=OP.is_lt,
                )
                t16 = small.tile([AH, 1], f32, tag="t16")
                nc.vector.scalar_tensor_tensor(
                    out=t16[:], in0=w[:], scalar=16.0, in1=r4[:],
                    op0=OP.mult, op1=OP.add,
                )
                pos = small.tile([AH, 1], f32, tag="ps")
                nc.vector.tensor_sub(pos[:], t16[:], parc_t[:])
                # id one-hot [i, g] and (pos one-hot * index) [i, k]
                a4 = small.tile([AH, 64], f32, tag="a4")
                nc.vector.tensor_scalar(
                    out=a4[:], in0=ps_idx[:, 0:64], scalar1=idc, scalar2=None,
                    op0=OP.is_equal,
                )
                oh = small.tile([AH, KMAX], f32, tag="oh")
                nc.vector.tensor_scalar(
                    out=oh[:], in0=ps_idx[:, 0:KMAX], scalar1=pos[:, 0:1],
                    scalar2=None, op0=OP.is_equal,
                )
                bv = small.tile([AH, KMAX], f32, tag="bv")
                nc.vector.tensor_scalar(
                    out=bv[:], in0=oh[:], scalar1=ixc, scalar2=None, op0=OP.mult,
                )
                nc.tensor.matmul(
                    ps_mt[:], a4[:], bv[:], start=(c4 == 0), stop=(c4 == 3)
                )
            m_sb = sb.tile([64, KMAX], f32)
            nc.scalar.copy(m_sb[:], ps_mt[:])

            # ptab[a,:] = M[id_a,:] via one-hot over g (K=64 matmul)
            ps_ohb = psrow.tile([64, AH], f32, tag="ohb")
            nc.tensor.matmul(
                ps_ohb[:], ones1[0:1, 0:64], idsAr_t[:], start=True, stop=True
            )
            ohT = sb.tile([64, AH], f32)
            nc.vector.tensor_scalar(
                out=ohT[:], in0=ps_ohb[:], scalar1=gcol_t[:, 0:1], scalar2=None,
                op0=OP.is_equal,
            )
            ps_ptab = psrow.tile([AH, KMAX], f32, tag="ptab")
            nc.tensor.matmul(ps_ptab[:], ohT[:], m_sb[:], start=True, stop=True)
            pf = sb.tile([AH, KMAX], f32)
            nc.scalar.copy(pf[:], ps_ptab[:])
            selfm = sb.tile([AH, KP], f32)
            nc.vector.tensor_scalar(
                out=selfm[:], in0=pf[:, 0:KP], scalar1=colsA_t[:, 1:2], scalar2=None,
                op0=OP.is_equal,
            )
            kv = sb.tile([AH, KP], f32)
            nc.vector.tensor_scalar(
                out=kv[:], in0=kidx_t[:], scalar1=cA[:], scalar2=None, op0=OP.is_lt,
            )
            vm = sb.tile([AH, KP], f32)
            nc.vector.tensor_sub(vm[:], kv[:], selfm[:])

            # ---- fetch d[a, p] for every member column in one gather
            pfs = sb.tile([AH, KP], f32)
            nc.vector.tensor_scalar(
                out=pfs[:], in0=pf[:, 0:KP], scalar1=rowb_t[:, 0:1], scalar2=None,
                op0=OP.add,
            )
            offi = sb.tile([AH, KP], i32)
            nc.vector.tensor_copy(offi[:], pfs[:])
            xg = sb.tile([AH, KP], f32)
            xall = sb.tile([AH, KP], f32)

            # ---- main loop over KP member columns (gather -> mask -> ops
            # per column so the pipeline fills column by column)
            ps_relu = psacc.tile([1, B], f32)
            ps_cnt = psacc.tile([1, B], f32)

            for j in range(KP):
                nc.gpsimd.indirect_dma_start(
                    out=xg[:, j : j + 1], out_offset=None, in_=dchd[:],
                    in_offset=IOA(ap=offi[:, j : j + 1], axis=0),
                )
                djm = small.tile([AH, 1], f32, tag="djm")
                nc.vector.tensor_scalar_add(djm[:], xg[:, j : j + 1], MARGIN)
                nc.vector.tensor_mul(xall[:, j : j + 1], djm[:], vm[:, j : j + 1])
                xj = xall[:, j : j + 1]
                t = junka.tile([AH, B], bf16)
                nc.scalar.activation(t[:], dneg[:], AF.Relu, bias=xj[:], scale=-1.0)
                nc.tensor.matmul(
                    ps_relu[:], ones128b[:], t[:],
                    start=(j == 0), stop=(j == KP - 1),
                )
                g = junkc.tile([AH, B], bf16)
                nc.vector.tensor_scalar(
                    out=g[:], in0=dneg_b[:], scalar1=xj[:], scalar2=None, op0=OP.is_lt,
                )
                nc.tensor.matmul(
                    ps_cnt[:], ones128b[:], g[:],
                    start=(j == 0), stop=(j == KP - 1),
                )

            # ---- final
            res = sb.tile([1, 2], f32)
            nc.vector.reduce_sum(res[:, 0:1], ps_relu[:], axis=X)
            nc.vector.reduce_sum(res[:, 1:2], ps_cnt[:], axis=X)
            nc.sync.dma_start(out=out[:], in_=res[:])

    return nc


def _legalize_waits(bir: bytes) -> bytes:
    """walrus codegen in this toolchain allows only one sync-wait per
    instruction; split extra waits into standalone EventSemaphore insts."""
    import json

    m = json.loads(bir)
    for fn in m["functions"]:
        for bb in fn["blocks"]:
            new = []
            for inst in bb["instructions"]:
                si = inst.get("sync_info")
                if si and si.get("on_wait") and len(si["on_wait"]) > 1:
                    waits = si["on_wait"]
                    for j, w in enumerate(waits[:-1]):
                        new.append(
                            {
                                "engine": inst["engine"],
                                "ins": [],
                                "outs": [],
                                "name": f"{inst['name']}-w{j}",
                                "opcode": "EventSemaphore",
                                "sync_info": {"on_update": [], "on_wait": [w]},
                            }
                        )
                    si["on_wait"] = [waits[-1]]
                new.append(inst)
            bb["instructions"] = new
    return json.dumps(m).encode()


def _get_nc():
    if "nc" not in _CACHE:
        nc = _build_bass()
        orig = nc.to_json_bytes
        nc.to_json_bytes = lambda: _legalize_waits(orig())
        _CACHE["nc"] = nc
    return _CACHE["nc"]


def make_in_maps(embs: np.ndarray, idtys: np.ndarray):
    embs = np.ascontiguousarray(np.asarray(embs, dtype=np.float32))
    emT = np.ascontiguousarray(embs.T)  # [D, B]
    ids = np.asarray(idtys).astype(np.float32)
    idx = np.arange(B, dtype=np.float32)
    in_maps = []
    for c in range(NCORES):
        a0 = (c // 2) * AH
        par = c % 2
        rows = np.concatenate([ids, idx])[None, :]
        colsA = np.stack([ids[a0 : a0 + AH], idx[a0 : a0 + AH]], axis=1)
        kcol = (np.arange(KP, dtype=np.float32) + 8.0 * par)[None, :]
        in_maps.append(
            {
                "emT": emT,
                "emTA": np.ascontiguousarray(emT[:, a0 : a0 + AH]),
                "rowb": (np.arange(AH, dtype=np.float32) * B).reshape(AH, 1),
                "rows": np.ascontiguousarray(rows.astype(np.float32)),
                "colsA": np.ascontiguousarray(colsA.astype(np.float32)),
                "idsAll": np.ascontiguousarray(ids.reshape(4, AH).T),
                "idxAll": np.ascontiguousarray(idx.reshape(4, AH).T),
                "kidx": np.ascontiguousarray(np.repeat(kcol, AH, axis=0)),
                "parc": np.full((AH, 1), 8.0 * par, dtype=np.float32),
                "idsAr": np.ascontiguousarray(ids[a0 : a0 + AH][None, :]),
                "gcol": np.arange(64, dtype=np.float32).reshape(64, 1),
            }
        )
    return in_maps


def combine(results):
    total = 0.0
    count = 0.0
    for r in results:
        o = np.asarray(r["out"], dtype=np.float64)
        total += o[0, 0]
        count += o[0, 1]
    loss = np.float32(total / (count + 1e-16))
    return np.array(loss, dtype=np.float32)


def kernel(embs: np.ndarray, idtys: np.ndarray) -> np.ndarray:
    from concourse import bass_utils

    nc = _get_nc()
    in_maps = make_in_maps(np.asarray(embs), np.asarray(idtys))
    res = bass_utils.run_bass_kernel_spmd(nc, in_maps, list(range(NCORES)))
    return combine(res.results)


# revision 20
# speedup vs baseline: 1.2123x; 1.0408x over previous
"""BatchAllTripletLoss on 8 Trainium2 NeuronCores (sparsity version).

Contract: kernel(**inputs) takes the FULL inputs (embs [512,128] f32,
idtys [512] int64) and returns the FULL output (scalar f32 loss).

Math: d = pairwise euclidean distances [512,512];
  loss = sum_{a,p,n} relu(d[a,p]-d[a,n]+margin)*mask / (num_pos + eps)
The triplet mask factorizes as pos[a,p]*neg[a,n] (pos: same id, p!=a;
neg: different id). With 64 ids over 512 samples, each anchor has only
~8 valid positives, so instead of brute-forcing all 512 p columns we
enumerate, per anchor, the members of its id group (ranked by a
device-side counting argsort) and only process those columns:

 1. d rows for this core's 128 anchors via PE matmul (+sq rank-1 folds),
    dneg = d + BIG*same  (neg mask folded; pushes relu/count to 0).
 2. Group member table: rank R_i = #(j<i with id_j==id_i) via a fused
    is_lt*same row-reduce; scatter index i into a DRAM table at row
    id_i*32 + perm(R_i) (indirect DMA). perm rotates ranks so that THIS
    core's parity class (R%2 == core parity) lands in columns 0..15 --
    per-core variation rides in input data, the program stays SPMD.
 3. Gather each anchor's member row [128,32] (indirect DMA by id), then
    per k-column: gather member embeddings [128,128], rowdot -> d[a,p]
    via sqrt(sqA+sqP-2dot), x = (d+margin)*(valid & p!=a).
 4. Main loop over just 16 k-columns (vs 512 brute-force):
    ACT: t = relu(x - y) bf16; DVE: g = 1[y_bf16 < x] bf16; the PE
    reduces every tile with ones[128,1] matmuls accumulated into PSUM.
Per-core output [1,2] = (relu sum, count); host sums cores and divides.
"""

import numpy as np

B = 512
D = 128
NCORES = 8
AH = 128          # anchors per core
KMAX = 16         # member-table width (max group size supported)
KP = 8            # k-columns processed per core (rank-half split of KMAX)
MARGIN = 0.2
BIG = 1.0e6

_CACHE = {}


def _build_bass():
    import concourse.bass as bass
    import concourse.tile as tile
    from concourse import mybir

    f32 = mybir.dt.float32
    i32 = mybir.dt.int32
    bf16 = mybir.dt.bfloat16
    AF = mybir.ActivationFunctionType
    OP = mybir.AluOpType
    IOA = bass.IndirectOffsetOnAxis
    X = mybir.AxisListType.X

    nc = bass.Bass()

    emT = nc.dram_tensor("emT", [D, B], f32, kind="ExternalInput")     # embs.T
    emTA = nc.dram_tensor("emTA", [D, AH], f32, kind="ExternalInput")  # anchor cols
    rows = nc.dram_tensor("rows", [1, 2 * B], f32, kind="ExternalInput")  # [ids|idx]
    # cols = [idsA, idxA, idsAll(4), idxAll(4), kidx(KP), parc, rowb, gcol]
    cols = nc.dram_tensor("cols", [AH, 13 + KP], f32, kind="ExternalInput")
    idsAr = nc.dram_tensor("idsAr", [1, AH], f32, kind="ExternalInput")  # ids of anchors, row
    out = nc.dram_tensor("out", [1, 2], f32, kind="ExternalOutput")

    dchd = nc.dram_tensor("dchd", [AH * B, 1], f32)      # d rows staged for gather

    with tile.TileContext(nc) as tc:
        with (
            tc.tile_pool(name="sb", bufs=1) as sb,
            tc.tile_pool(name="psrow", bufs=1, space="PSUM") as psrow,
            tc.tile_pool(name="psbig", bufs=2, space="PSUM") as psbig,
            tc.tile_pool(name="psacc", bufs=1, space="PSUM") as psacc,
            tc.tile_pool(name="junka", bufs=4) as junka,
            tc.tile_pool(name="junkc", bufs=4) as junkc,
            tc.tile_pool(name="small", bufs=4) as small,
            tc.tile_pool(name="emb", bufs=4) as emb,
        ):
            # ---- load inputs
            emT_t = sb.tile([D, B], f32)
            emTA_t = sb.tile([D, AH], f32)
            rows_t = sb.tile([1, 2 * B], f32)
            cols_t = sb.tile([AH, 13 + KP], f32)
            idsAr_t = sb.tile([1, AH], f32)
            nc.sync.dma_start(out=emT_t[:], in_=emT[:])
            nc.sync.dma_start(out=emTA_t[:], in_=emTA[:])
            nc.sync.dma_start(out=rows_t[:], in_=rows[:])
            nc.sync.dma_start(out=cols_t[:], in_=cols[:])
            nc.sync.dma_start(out=idsAr_t[:], in_=idsAr[:])
            colsA_t = cols_t[:, 0:2]
            idsAll_t = cols_t[:, 2:6]
            idxAll_t = cols_t[:, 6:10]
            kidx_t = cols_t[:, 10 : 10 + KP]
            parc_t = cols_t[:, 10 + KP : 11 + KP]
            rowb_t = cols_t[:, 11 + KP : 12 + KP]
            gcol_t = cols_t[0:64, 12 + KP : 13 + KP]

            ones128 = sb.tile([D, 1], f32)
            nc.vector.memset(ones128[:], 1.0)
            ones128b = sb.tile([D, 1], bf16)
            nc.vector.memset(ones128b[:], 1.0)
            ones1 = sb.tile([1, D], f32)
            nc.vector.memset(ones1[:], 1.0)
            ones_row = sb.tile([1, B], f32)
            nc.vector.memset(ones_row[:], 1.0)

            # ---- squared norms
            sq_sb = sb.tile([1, B], f32)
            sqa_sb = sb.tile([1, AH], f32)
            e2 = sb.tile([D, B], f32)
            nc.vector.tensor_mul(e2[:], emT_t[:], emT_t[:])
            ps_sq = psrow.tile([1, B], f32, tag="row")
            nc.tensor.matmul(ps_sq[:], ones128[:], e2[:], start=True, stop=True)
            nc.scalar.copy(sq_sb[:], ps_sq[:])
            e2a = sb.tile([D, AH], f32)
            nc.vector.tensor_mul(e2a[:], emTA_t[:], emTA_t[:])
            ps_sqa = psrow.tile([1, AH], f32, tag="row")
            nc.tensor.matmul(ps_sqa[:], ones128[:], e2a[:], start=True, stop=True)
            nc.scalar.copy(sqa_sb[:], ps_sqa[:])

            emTAm2 = sb.tile([D, AH], f32)
            nc.vector.tensor_scalar_mul(emTAm2[:], emTA_t[:], -2.0)

            # d2 rows for this core's anchors, full n range
            ps_d2 = psbig.tile([AH, B], f32, tag="big")
            nc.tensor.matmul(ps_d2[:], emTAm2[:], emT_t[:], start=True, stop=False)
            nc.tensor.matmul(ps_d2[:], sqa_sb[:], ones_row[:], start=False, stop=False)
            nc.tensor.matmul(ps_d2[:], ones1[:, 0:AH], sq_sb[:], start=False, stop=True)
            d2r = sb.tile([AH, B], f32)
            nc.vector.tensor_scalar_max(d2r[:], ps_d2[:], 0.0)
            dch = sb.tile([AH, B], f32)
            nc.scalar.activation(dch[:], d2r[:], AF.Sqrt)
            dchd_v = dchd[:].rearrange("(a b) one -> a (b one)", a=AH)
            nc.sync.dma_start(out=dchd_v, in_=dch[:])

            # id/idx broadcast rows (persist through the scatter loop)
            ps_ids = psbig.tile([AH, B], f32, tag="big")
            nc.tensor.matmul(ps_ids[:], ones1[:], rows_t[0:1, 0:B], start=True, stop=True)
            ps_idx = psbig.tile([AH, B], f32, tag="big")
            nc.tensor.matmul(
                ps_idx[:], ones1[:], rows_t[0:1, B : 2 * B], start=True, stop=True
            )

            # dneg = d + BIG*same; group size cA per anchor
            s_full = sb.tile([AH, B], f32)
            nc.vector.tensor_scalar(
                out=s_full[:], in0=ps_ids[:], scalar1=colsA_t[:, 0:1], scalar2=None,
                op0=OP.is_equal,
            )
            dneg = sb.tile([AH, B], f32)
            nc.vector.scalar_tensor_tensor(
                out=dneg[:], in0=s_full[:], scalar=BIG, in1=dch[:],
                op0=OP.mult, op1=OP.add,
            )
            dneg_b = sb.tile([AH, B], bf16)
            nc.vector.tensor_copy(dneg_b[:], dneg[:])
            cA = sb.tile([AH, 1], f32)
            nc.vector.reduce_sum(cA[:], s_full[:], axis=X)

            # ---- build member table M[g,k] = sum_i id-onehot * pos-onehot * i
            # via PE matmuls over 4 chunks of i, then ptab[a,:] = M[id_a,:]
            # via a second one-hot matmul. No indirect DMA needed.
            ps_mt = psrow.tile([64, KMAX], f32, tag="mt")
            for c4 in range(4):
                idc = idsAll_t[:, c4 : c4 + 1]
                ixc = idxAll_t[:, c4 : c4 + 1]
                s4 = small.tile([AH, B], f32, tag="s4")
                nc.vector.tensor_scalar(
                    out=s4[:], in0=ps_ids[:], scalar1=idc, scalar2=None,
                    op0=OP.is_equal,
                )
                jl = small.tile([AH, B], f32, tag="jl")
                nc.vector.tensor_scalar(
                    out=jl[:], in0=ps_idx[:], scalar1=ixc, scalar2=None,
                    op0=OP.is_lt,
                )
                jm = small.tile([AH, B], f32, tag="jm")
                nc.vector.tensor_mul(jm[:], jl[:], s4[:])
                r4 = small.tile([AH, 1], f32, tag="r4")
                jr = small.tile([AH, B], f32, tag="jr")
                nc.vector.tensor_scalar(
                    out=jr[:], in0=jm[:], scalar1=1.0, scalar2=None,
                    op0=OP.mult, op1=OP.add, accum_out=r4[:],
                )
                # perm: pos = R - 8*par + 16*[R < 8*par] -- rotates this
                # core's rank half to cols 0..7, parks the rest in 8..15
                w = small.tile([AH, 1], f32, tag="w")
                nc.vector.tensor_tensor(
                    out=w[:], in0=r4[:], in1=parc_t[:], op=OP.is_lt,
                )
                t16 = small.tile([AH, 1], f32, tag="t16")
                nc.vector.scalar_tensor_tensor(
                    out=t16[:], in0=w[:], scalar=16.0, in1=r4[:],
                    op0=OP.mult, op1=OP.add,
                )
                pos = small.tile([AH, 1], f32, tag="ps")
                nc.vector.tensor_sub(pos[:], t16[:], parc_t[:])
                # id one-hot [i, g] and (pos one-hot * index) [i, k]
                a4 = small.tile([AH, 64], f32, tag="a4")
                nc.vector.tensor_scalar(
                    out=a4[:], in0=ps_idx[:, 0:64], scalar1=idc, scalar2=None,
                    op0=OP.is_equal,
                )
                oh = small.tile([AH, KMAX], f32, tag="oh")
                nc.vector.tensor_scalar(
                    out=oh[:], in0=ps_idx[:, 0:KMAX], scalar1=pos[:, 0:1],
                    scalar2=None, op0=OP.is_equal,
                )
                bv = small.tile([AH, KMAX], f32, tag="bv")
                nc.vector.tensor_scalar(
                    out=bv[:], in0=oh[:], scalar1=ixc, scalar2=None, op0=OP.mult,
                )
                nc.tensor.matmul(
                    ps_mt[:], a4[:], bv[:], start=(c4 == 0), stop=(c4 == 3)
                )
            m_sb = sb.tile([64, KMAX], f32)
            nc.scalar.copy(m_sb[:], ps_mt[:])

            # ptab[a,:] = M[id_a,:] via one-hot over g (K=64 matmul)
            ps_ohb = psrow.tile([64, AH], f32, tag="ohb")
            nc.tensor.matmul(
                ps_ohb[:], ones1[0:1, 0:64], idsAr_t[:], start=True, stop=True
            )
            ohT = sb.tile([64, AH], f32)
            nc.vector.tensor_scalar(
                out=ohT[:], in0=ps_ohb[:], scalar1=gcol_t[:, 0:1], scalar2=None,
                op0=OP.is_equal,
            )
            ps_ptab = psrow.tile([AH, KMAX], f32, tag="ptab")
            nc.tensor.matmul(ps_ptab[:], ohT[:], m_sb[:], start=True, stop=True)
            pf = sb.tile([AH, KMAX], f32)
            nc.scalar.copy(pf[:], ps_ptab[:])
            selfm = sb.tile([AH, KP], f32)
            nc.vector.tensor_scalar(
                out=selfm[:], in0=pf[:, 0:KP], scalar1=colsA_t[:, 1:2], scalar2=None,
                op0=OP.is_equal,
            )
            kv = sb.tile([AH, KP], f32)
            nc.vector.tensor_scalar(
                out=kv[:], in0=kidx_t[:], scalar1=cA[:], scalar2=None, op0=OP.is_lt,
            )
            vm = sb.tile([AH, KP], f32)
            nc.vector.tensor_sub(vm[:], kv[:], selfm[:])

            # ---- fetch d[a, p] for every member column in one gather
            pfs = sb.tile([AH, KP], f32)
            nc.vector.tensor_scalar(
                out=pfs[:], in0=pf[:, 0:KP], scalar1=rowb_t[:, 0:1], scalar2=None,
                op0=OP.add,
            )
            offi = sb.tile([AH, KP], i32)
            nc.vector.tensor_copy(offi[:], pfs[:])
            xg = sb.tile([AH, KP], f32)
            xall = sb.tile([AH, KP], f32)

            # ---- main loop over KP member columns (gather -> mask -> ops
            # per column so the pipeline fills column by column)
            ps_relu = psacc.tile([1, B], f32)
            ps_cnt = psacc.tile([1, B], f32)

            for j in range(KP):
                nc.gpsimd.indirect_dma_start(
                    out=xg[:, j : j + 1], out_offset=None, in_=dchd[:],
                    in_offset=IOA(ap=offi[:, j : j + 1], axis=0),
                )
                djm = small.tile([AH, 1], f32, tag="djm")
                nc.vector.tensor_scalar_add(djm[:], xg[:, j : j + 1], MARGIN)
                nc.vector.tensor_mul(xall[:, j : j + 1], djm[:], vm[:, j : j + 1])
                xj = xall[:, j : j + 1]
                t = junka.tile([AH, B], bf16)
                nc.scalar.activation(t[:], dneg[:], AF.Relu, bias=xj[:], scale=-1.0)
                nc.tensor.matmul(
                    ps_relu[:], ones128b[:], t[:],
                    start=(j == 0), stop=(j == KP - 1),
                )
                g = junkc.tile([AH, B], bf16)
                nc.vector.tensor_scalar(
                    out=g[:], in0=dneg_b[:], scalar1=xj[:], scalar2=None, op0=OP.is_lt,
                )
                nc.tensor.matmul(
                    ps_cnt[:], ones128b[:], g[:],
                    start=(j == 0), stop=(j == KP - 1),
                )

            # ---- final
            res = sb.tile([1, 2], f32)
            nc.vector.reduce_sum(res[:, 0:1], ps_relu[:], axis=X)
            nc.vector.reduce_sum(res[:, 1:2], ps_cnt[:], axis=X)
            nc.sync.dma_start(out=out[:], in_=res[:])

    return nc


def _legalize_waits(bir: bytes) -> bytes:
    """walrus codegen in this toolchain allows only one sync-wait per
    instruction; split extra waits into standalone EventSemaphore insts."""
    import json

    m = json.loads(bir)
    for fn in m["functions"]:
        for bb in fn["blocks"]:
            new = []
            for inst in bb["instructions"]:
                si = inst.get("sync_info")
                if si and si.get("on_wait") and len(si["on_wait"]) > 1:
                    waits = si["on_wait"]
                    for j, w in enumerate(waits[:-1]):
                        new.append(
                            {
                                "engine": inst["engine"],
                                "ins": [],
                                "outs": [],
                                "name": f"{inst['name']}-w{j}",
                                "opcode": "EventSemaphore",
                                "sync_info": {"on_update": [], "on_wait": [w]},
                            }
                        )
                    si["on_wait"] = [waits[-1]]
                new.append(inst)
            bb["instructions"] = new
    return json.dumps(m).encode()


def _get_nc():
    if "nc" not in _CACHE:
        nc = _build_bass()
        orig = nc.to_json_bytes
        nc.to_json_bytes = lambda: _legalize_waits(orig())
        _CACHE["nc"] = nc
    return _CACHE["nc"]


def make_in_maps(embs: np.ndarray, idtys: np.ndarray):
    embs = np.ascontiguousarray(np.asarray(embs, dtype=np.float32))
    emT = np.ascontiguousarray(embs.T)  # [D, B]
    ids = np.asarray(idtys).astype(np.float32)
    idx = np.arange(B, dtype=np.float32)
    in_maps = []
    for c in range(NCORES):
        a0 = (c // 2) * AH
        par = c % 2
        rows = np.concatenate([ids, idx])[None, :]
        kcol = (np.arange(KP, dtype=np.float32) + 8.0 * par)[None, :]
        gc = np.zeros((AH, 1), dtype=np.float32)
        gc[:64, 0] = np.arange(64, dtype=np.float32)
        cols = np.concatenate(
            [
                ids[a0 : a0 + AH].reshape(AH, 1),
                idx[a0 : a0 + AH].reshape(AH, 1),
                ids.reshape(4, AH).T,
                idx.reshape(4, AH).T,
                np.repeat(kcol, AH, axis=0),
                np.full((AH, 1), 8.0 * par, dtype=np.float32),
                (np.arange(AH, dtype=np.float32) * B).reshape(AH, 1),
                gc,
            ],
            axis=1,
        ).astype(np.float32)
        in_maps.append(
            {
                "emT": emT,
                "emTA": np.ascontiguousarray(emT[:, a0 : a0 + AH]),
                "rows": np.ascontiguousarray(rows.astype(np.float32)),
                "cols": np.ascontiguousarray(cols),
                "idsAr": np.ascontiguousarray(ids[a0 : a0 + AH][None, :]),
            }
        )
    return in_maps


def combine(results):
    total = 0.0
    count = 0.0
    for r in results:
        o = np.asarray(r["out"], dtype=np.float64)
        total += o[0, 0]
        count += o[0, 1]
    loss = np.float32(total / (count + 1e-16))
    return np.array(loss, dtype=np.float32)


def kernel(embs: np.ndarray, idtys: np.ndarray) -> np.ndarray:
    from concourse import bass_utils

    nc = _get_nc()
    in_maps = make_in_maps(np.asarray(embs), np.asarray(idtys))
    res = bass_utils.run_bass_kernel_spmd(nc, in_maps, list(range(NCORES)))
    return combine(res.results)


# revision 21
# speedup vs baseline: 1.3263x; 1.0941x over previous
"""BatchAllTripletLoss on 8 Trainium2 NeuronCores (sparsity version).

Contract: kernel(**inputs) takes the FULL inputs (embs [512,128] f32,
idtys [512] int64) and returns the FULL output (scalar f32 loss).

Math: d = pairwise euclidean distances [512,512];
  loss = sum_{a,p,n} relu(d[a,p]-d[a,n]+margin)*mask / (num_pos + eps)
The triplet mask factorizes as pos[a,p]*neg[a,n] (pos: same id, p!=a;
neg: different id). With 64 ids over 512 samples, each anchor has only
~8 valid positives, so instead of brute-forcing all 512 p columns we
enumerate, per anchor, the members of its id group (ranked by a
device-side counting argsort) and only process those columns:

 1. d rows for this core's 128 anchors via PE matmul (+sq rank-1 folds),
    dneg = d + BIG*same  (neg mask folded; pushes relu/count to 0).
 2. Group member table: rank R_i = #(j<i with id_j==id_i) via a fused
    is_lt*same row-reduce; scatter index i into a DRAM table at row
    id_i*32 + perm(R_i) (indirect DMA). perm rotates ranks so that THIS
    core's parity class (R%2 == core parity) lands in columns 0..15 --
    per-core variation rides in input data, the program stays SPMD.
 3. Gather each anchor's member row [128,32] (indirect DMA by id), then
    per k-column: gather member embeddings [128,128], rowdot -> d[a,p]
    via sqrt(sqA+sqP-2dot), x = (d+margin)*(valid & p!=a).
 4. Main loop over just 16 k-columns (vs 512 brute-force):
    ACT: t = relu(x - y) bf16; DVE: g = 1[y_bf16 < x] bf16; the PE
    reduces every tile with ones[128,1] matmuls accumulated into PSUM.
Per-core output [1,2] = (relu sum, count); host sums cores and divides.
"""

import numpy as np

B = 512
D = 128
NCORES = 8
AH = 128          # anchors per core
KMAX = 16         # member-table width (max group size supported)
KP = 8            # k-columns processed per core (rank-half split of KMAX)
MARGIN = 0.2
BIG = 1.0e6

_CACHE = {}


def _build_bass():
    import concourse.bass as bass
    import concourse.tile as tile
    from concourse import mybir

    f32 = mybir.dt.float32
    i32 = mybir.dt.int32
    bf16 = mybir.dt.bfloat16
    AF = mybir.ActivationFunctionType
    OP = mybir.AluOpType
    IOA = bass.IndirectOffsetOnAxis
    X = mybir.AxisListType.X

    nc = bass.Bass()

    emT = nc.dram_tensor("emT", [D, B], f32, kind="ExternalInput")     # embs.T
    emTA = nc.dram_tensor("emTA", [D, AH], f32, kind="ExternalInput")  # anchor cols
    rows = nc.dram_tensor("rows", [1, 2 * B], f32, kind="ExternalInput")  # [ids|idx]
    # cols = [idsA, idxA, idsAll(4), idxAll(4), kidx(KP), parc, rowb, gcol]
    cols = nc.dram_tensor("cols", [AH, 13 + KP], f32, kind="ExternalInput")
    idsAr = nc.dram_tensor("idsAr", [1, AH], f32, kind="ExternalInput")  # ids of anchors, row
    out = nc.dram_tensor("out", [1, 2], f32, kind="ExternalOutput")

    dchd = nc.dram_tensor("dchd", [AH * B, 1], f32)      # d rows staged for gather

    with tile.TileContext(nc) as tc:
        with (
            tc.tile_pool(name="sb", bufs=1) as sb,
            tc.tile_pool(name="psrow", bufs=1, space="PSUM") as psrow,
            tc.tile_pool(name="psbig", bufs=2, space="PSUM") as psbig,
            tc.tile_pool(name="psacc", bufs=1, space="PSUM") as psacc,
            tc.tile_pool(name="junka", bufs=4) as junka,
            tc.tile_pool(name="junkc", bufs=4) as junkc,
            tc.tile_pool(name="small", bufs=4) as small,
            tc.tile_pool(name="emb", bufs=4) as emb,
        ):
            # ---- load inputs
            emT_t = sb.tile([D, B], f32)
            emTA_t = sb.tile([D, AH], f32)
            rows_t = sb.tile([1, 2 * B], f32)
            cols_t = sb.tile([AH, 13 + KP], f32)
            idsAr_t = sb.tile([1, AH], f32)
            nc.sync.dma_start(out=emT_t[:], in_=emT[:])
            nc.sync.dma_start(out=emTA_t[:], in_=emTA[:])
            nc.sync.dma_start(out=rows_t[:], in_=rows[:])
            nc.sync.dma_start(out=cols_t[:], in_=cols[:])
            nc.sync.dma_start(out=idsAr_t[:], in_=idsAr[:])
            colsA_t = cols_t[:, 0:2]
            idsAll_t = cols_t[:, 2:6]
            idxAll_t = cols_t[:, 6:10]
            kidx_t = cols_t[:, 10 : 10 + KP]
            parc_t = cols_t[:, 10 + KP : 11 + KP]
            rowb_t = cols_t[:, 11 + KP : 12 + KP]
            gcol_t = cols_t[0:64, 12 + KP : 13 + KP]

            ones128 = sb.tile([D, 1], f32)
            nc.vector.memset(ones128[:], 1.0)
            ones128b = sb.tile([D, 1], bf16)
            nc.vector.memset(ones128b[:], 1.0)
            ones1 = sb.tile([1, D], f32)
            nc.vector.memset(ones1[:], 1.0)
            ones_row = sb.tile([1, B], f32)
            nc.vector.memset(ones_row[:], 1.0)

            # ---- squared norms
            sq_sb = sb.tile([1, B], f32)
            sqa_sb = sb.tile([1, AH], f32)
            e2 = sb.tile([D, B], f32)
            nc.vector.tensor_mul(e2[:], emT_t[:], emT_t[:])
            ps_sq = psrow.tile([1, B], f32, tag="row")
            nc.tensor.matmul(ps_sq[:], ones128[:], e2[:], start=True, stop=True)
            nc.scalar.copy(sq_sb[:], ps_sq[:])
            e2a = sb.tile([D, AH], f32)
            nc.vector.tensor_mul(e2a[:], emTA_t[:], emTA_t[:])
            ps_sqa = psrow.tile([1, AH], f32, tag="row")
            nc.tensor.matmul(ps_sqa[:], ones128[:], e2a[:], start=True, stop=True)
            nc.scalar.copy(sqa_sb[:], ps_sqa[:])

            emTAm2 = sb.tile([D, AH], f32)
            nc.vector.tensor_scalar_mul(emTAm2[:], emTA_t[:], -2.0)

            # d2 rows for this core's anchors, full n range
            ps_d2 = psbig.tile([AH, B], f32, tag="big")
            nc.tensor.matmul(ps_d2[:], emTAm2[:], emT_t[:], start=True, stop=False)
            nc.tensor.matmul(ps_d2[:], sqa_sb[:], ones_row[:], start=False, stop=False)
            nc.tensor.matmul(ps_d2[:], ones1[:, 0:AH], sq_sb[:], start=False, stop=True)
            d2r = sb.tile([AH, B], f32)
            nc.vector.tensor_scalar_max(d2r[:], ps_d2[:], 0.0)
            dch = sb.tile([AH, B], f32)
            nc.scalar.activation(dch[:], d2r[:], AF.Sqrt)
            dchd_v = dchd[:].rearrange("(a b) one -> a (b one)", a=AH)
            nc.sync.dma_start(out=dchd_v, in_=dch[:])

            # id/idx broadcast rows (persist through the scatter loop)
            ps_ids = psbig.tile([AH, B], f32, tag="big")
            nc.tensor.matmul(ps_ids[:], ones1[:], rows_t[0:1, 0:B], start=True, stop=True)
            ps_idx = psbig.tile([AH, B], f32, tag="big")
            nc.tensor.matmul(
                ps_idx[:], ones1[:], rows_t[0:1, B : 2 * B], start=True, stop=True
            )
            # copy broadcasts to SBUF so DVE readers get 2x mode (PSUM src
            # is capped at 1x with a 120-cycle init)
            ids_sb = sb.tile([AH, B], f32)
            nc.scalar.copy(ids_sb[:], ps_ids[:])
            idx_sb = sb.tile([AH, B], f32)
            nc.scalar.copy(idx_sb[:], ps_idx[:])

            # dneg = d + BIG*same; group size cA per anchor
            s_full = sb.tile([AH, B], f32)
            nc.vector.tensor_scalar(
                out=s_full[:], in0=ids_sb[:], scalar1=colsA_t[:, 0:1], scalar2=None,
                op0=OP.is_equal,
            )
            dneg = sb.tile([AH, B], f32)
            nc.vector.scalar_tensor_tensor(
                out=dneg[:], in0=s_full[:], scalar=BIG, in1=dch[:],
                op0=OP.mult, op1=OP.add,
            )
            dneg_b = sb.tile([AH, B], bf16)
            nc.vector.tensor_copy(dneg_b[:], dneg[:])
            cA = sb.tile([AH, 1], f32)
            nc.vector.reduce_sum(cA[:], s_full[:], axis=X)

            # ---- build member table M[g,k] = sum_i id-onehot * pos-onehot * i
            # via PE matmuls over 4 chunks of i, then ptab[a,:] = M[id_a,:]
            # via a second one-hot matmul. No indirect DMA needed.
            ps_mt = psrow.tile([64, KMAX], f32, tag="mt")
            for c4 in range(4):
                idc = idsAll_t[:, c4 : c4 + 1]
                ixc = idxAll_t[:, c4 : c4 + 1]
                s4 = small.tile([AH, B], f32, tag="s4")
                nc.vector.tensor_scalar(
                    out=s4[:], in0=ids_sb[:], scalar1=idc, scalar2=None,
                    op0=OP.is_equal,
                )
                jl = small.tile([AH, B], f32, tag="jl")
                nc.vector.tensor_scalar(
                    out=jl[:], in0=idx_sb[:], scalar1=ixc, scalar2=None,
                    op0=OP.is_lt,
                )
                jm = small.tile([AH, B], f32, tag="jm")
                nc.vector.tensor_mul(jm[:], jl[:], s4[:])
                r4 = small.tile([AH, 1], f32, tag="r4")
                jr = small.tile([AH, B], f32, tag="jr")
                nc.vector.tensor_scalar(
                    out=jr[:], in0=jm[:], scalar1=1.0, scalar2=None,
                    op0=OP.mult, op1=OP.add, accum_out=r4[:],
                )
                # perm: pos = R - 8*par + 16*[R < 8*par] -- rotates this
                # core's rank half to cols 0..7, parks the rest in 8..15
                w = small.tile([AH, 1], f32, tag="w")
                nc.vector.tensor_tensor(
                    out=w[:], in0=r4[:], in1=parc_t[:], op=OP.is_lt,
                )
                t16 = small.tile([AH, 1], f32, tag="t16")
                nc.vector.scalar_tensor_tensor(
                    out=t16[:], in0=w[:], scalar=16.0, in1=r4[:],
                    op0=OP.mult, op1=OP.add,
                )
                pos = small.tile([AH, 1], f32, tag="ps")
                nc.vector.tensor_sub(pos[:], t16[:], parc_t[:])
                # id one-hot [i, g] and (pos one-hot * index) [i, k]
                a4 = small.tile([AH, 64], f32, tag="a4")
                nc.vector.tensor_scalar(
                    out=a4[:], in0=idx_sb[:, 0:64], scalar1=idc, scalar2=None,
                    op0=OP.is_equal,
                )
                oh = small.tile([AH, KMAX], f32, tag="oh")
                nc.vector.tensor_scalar(
                    out=oh[:], in0=idx_sb[:, 0:KMAX], scalar1=pos[:, 0:1],
                    scalar2=None, op0=OP.is_equal,
                )
                bv = small.tile([AH, KMAX], f32, tag="bv")
                nc.vector.tensor_scalar(
                    out=bv[:], in0=oh[:], scalar1=ixc, scalar2=None, op0=OP.mult,
                )
                nc.tensor.matmul(
                    ps_mt[:], a4[:], bv[:], start=(c4 == 0), stop=(c4 == 3)
                )
            m_sb = sb.tile([64, KMAX], f32)
            nc.scalar.copy(m_sb[:], ps_mt[:])

            # ptab[a,:] = M[id_a,:] via one-hot over g (K=64 matmul)
            ps_ohb = psrow.tile([64, AH], f32, tag="ohb")
            nc.tensor.matmul(
                ps_ohb[:], ones1[0:1, 0:64], idsAr_t[:], start=True, stop=True
            )
            ohT = sb.tile([64, AH], f32)
            nc.vector.tensor_scalar(
                out=ohT[:], in0=ps_ohb[:], scalar1=gcol_t[:, 0:1], scalar2=None,
                op0=OP.is_equal,
            )
            ps_ptab = psrow.tile([AH, KMAX], f32, tag="ptab")
            nc.tensor.matmul(ps_ptab[:], ohT[:], m_sb[:], start=True, stop=True)
            pf = sb.tile([AH, KMAX], f32)
            nc.scalar.copy(pf[:], ps_ptab[:])
            selfm = sb.tile([AH, KP], f32)
            nc.vector.tensor_scalar(
                out=selfm[:], in0=pf[:, 0:KP], scalar1=colsA_t[:, 1:2], scalar2=None,
                op0=OP.is_equal,
            )
            kv = sb.tile([AH, KP], f32)
            nc.vector.tensor_scalar(
                out=kv[:], in0=kidx_t[:], scalar1=cA[:], scalar2=None, op0=OP.is_lt,
            )
            vm = sb.tile([AH, KP], f32)
            nc.vector.tensor_sub(vm[:], kv[:], selfm[:])

            # ---- fetch d[a, p] for every member column in one gather
            pfs = sb.tile([AH, KP], f32)
            nc.vector.tensor_scalar(
                out=pfs[:], in0=pf[:, 0:KP], scalar1=rowb_t[:, 0:1], scalar2=None,
                op0=OP.add,
            )
            offi = sb.tile([AH, KP], i32)
            nc.vector.tensor_copy(offi[:], pfs[:])
            xg = sb.tile([AH, KP], f32)
            xall = sb.tile([AH, KP], f32)

            # ---- main loop over KP member columns (gather -> mask -> ops
            # per column so the pipeline fills column by column)
            ps_relu = psacc.tile([1, B], f32)
            ps_cnt = psacc.tile([1, B], f32)

            for j in range(KP):
                nc.gpsimd.indirect_dma_start(
                    out=xg[:, j : j + 1], out_offset=None, in_=dchd[:],
                    in_offset=IOA(ap=offi[:, j : j + 1], axis=0),
                )
                djm = small.tile([AH, 1], f32, tag="djm")
                nc.vector.tensor_scalar_add(djm[:], xg[:, j : j + 1], MARGIN)
                nc.vector.tensor_mul(xall[:, j : j + 1], djm[:], vm[:, j : j + 1])
                xj = xall[:, j : j + 1]
                t = junka.tile([AH, B], bf16)
                nc.scalar.activation(t[:], dneg[:], AF.Relu, bias=xj[:], scale=-1.0)
                nc.tensor.matmul(
                    ps_relu[:], ones128b[:], t[:],
                    start=(j == 0), stop=(j == KP - 1),
                )
                g = junkc.tile([AH, B], bf16)
                nc.vector.tensor_scalar(
                    out=g[:], in0=dneg_b[:], scalar1=xj[:], scalar2=None, op0=OP.is_lt,
                )
                nc.tensor.matmul(
                    ps_cnt[:], ones128b[:], g[:],
                    start=(j == 0), stop=(j == KP - 1),
                )

            # ---- final
            res = sb.tile([1, 2], f32)
            nc.vector.reduce_sum(res[:, 0:1], ps_relu[:], axis=X)
            nc.vector.reduce_sum(res[:, 1:2], ps_cnt[:], axis=X)
            nc.sync.dma_start(out=out[:], in_=res[:])

    return nc


def _legalize_waits(bir: bytes) -> bytes:
    """walrus codegen in this toolchain allows only one sync-wait per
    instruction; split extra waits into standalone EventSemaphore insts."""
    import json

    m = json.loads(bir)
    for fn in m["functions"]:
        for bb in fn["blocks"]:
            new = []
            for inst in bb["instructions"]:
                si = inst.get("sync_info")
                if si and si.get("on_wait") and len(si["on_wait"]) > 1:
                    waits = si["on_wait"]
                    for j, w in enumerate(waits[:-1]):
                        new.append(
                            {
                                "engine": inst["engine"],
                                "ins": [],
                                "outs": [],
                                "name": f"{inst['name']}-w{j}",
                                "opcode": "EventSemaphore",
                                "sync_info": {"on_update": [], "on_wait": [w]},
                            }
                        )
                    si["on_wait"] = [waits[-1]]
                new.append(inst)
            bb["instructions"] = new
    return json.dumps(m).encode()


def _get_nc():
    if "nc" not in _CACHE:
        nc = _build_bass()
        orig = nc.to_json_bytes
        nc.to_json_bytes = lambda: _legalize_waits(orig())
        _CACHE["nc"] = nc
    return _CACHE["nc"]


def make_in_maps(embs: np.ndarray, idtys: np.ndarray):
    embs = np.ascontiguousarray(np.asarray(embs, dtype=np.float32))
    emT = np.ascontiguousarray(embs.T)  # [D, B]
    ids = np.asarray(idtys).astype(np.float32)
    idx = np.arange(B, dtype=np.float32)
    in_maps = []
    for c in range(NCORES):
        a0 = (c // 2) * AH
        par = c % 2
        rows = np.concatenate([ids, idx])[None, :]
        kcol = (np.arange(KP, dtype=np.float32) + 8.0 * par)[None, :]
        gc = np.zeros((AH, 1), dtype=np.float32)
        gc[:64, 0] = np.arange(64, dtype=np.float32)
        cols = np.concatenate(
            [
                ids[a0 : a0 + AH].reshape(AH, 1),
                idx[a0 : a0 + AH].reshape(AH, 1),
                ids.reshape(4, AH).T,
                idx.reshape(4, AH).T,
                np.repeat(kcol, AH, axis=0),
                np.full((AH, 1), 8.0 * par, dtype=np.float32),
                (np.arange(AH, dtype=np.float32) * B).reshape(AH, 1),
                gc,
            ],
            axis=1,
        ).astype(np.float32)
        in_maps.append(
            {
                "emT": emT,
                "emTA": np.ascontiguousarray(emT[:, a0 : a0 + AH]),
                "rows": np.ascontiguousarray(rows.astype(np.float32)),
                "cols": np.ascontiguousarray(cols),
                "idsAr": np.ascontiguousarray(ids[a0 : a0 + AH][None, :]),
            }
        )
    return in_maps


def combine(results):
    total = 0.0
    count = 0.0
    for r in results:
        o = np.asarray(r["out"], dtype=np.float64)
        total += o[0, 0]
        count += o[0, 1]
    loss = np.float32(total / (count + 1e-16))
    return np.array(loss, dtype=np.float32)


def kernel(embs: np.ndarray, idtys: np.ndarray) -> np.ndarray:
    from concourse import bass_utils

    nc = _get_nc()
    in_maps = make_in_maps(np.asarray(embs), np.asarray(idtys))
    res = bass_utils.run_bass_kernel_spmd(nc, in_maps, list(range(NCORES)))
    return combine(res.results)
